# revision 1
# baseline (speedup 1.0000x reference)
"""CrossBatchAttention Trainium2 kernel — 8-core tensor-parallel SPMD.

v2: fp8 DoubleRow everywhere (see v1) + software-pipelined schedule that
keeps the PE busy end-to-end:

  K-pass (all batch quarters) -> V+g1X-pass -> 16 interleaved blocks.
  Block s = (head h, batch-quarter q):
    - prefetch out_proj operand for the AllGather issued 2 blocks ago
    - attention: S^T j-tile pairs (bf16) -> paired Exp ACT out of 2-bank
      PSUM -> diagonal zero -> denominator pass (fp8 DR) -> O^T pass
      (fp8 DR) -> normalize -> fp8 otc -> per-(h,q) AllGather (64KB in)
    - out_proj group for the block-(s-2) AG chunk, accumulated into fp8
      cacc (cross^T)
    - Q-projection m-tile for quarter q+1 (the m-tile IS the head)
    - when a 512-query i-chunk's cacc completes: g1C partial (fp8 DR),
      inject the core's g1X shard into its own row-block, single fp8
      AllReduce (replaces ReduceScatter+AllGather); 3 blocks later: gelu
      + gW2 logits (fp8 DR) + sigmoid + gated output (bf16).
Host: concat 8 [512,2048] bf16 shards, transpose, add X -> f32.

Quantization: X fp8, W* fp8 x64 (/64 on PSUM exit), qt/kt bf16,
P = exp(s/sqrt(d) - 5.0) fp8 (score max ~10 for these inputs),
ones = 1/8 so rec = 8/den, otc = O*rec fp8 (std ~0.9), cacc fp8 natural
(/512 fold), g1 partials fp8 x8 for the AllReduce, sigmoid(logits/64).
"""

import numpy as np
import ml_dtypes

import concourse.bass as bass
import concourse.mybir as mybir
import concourse.tile as tile
from concourse import bacc
from concourse import bass_utils

BF16 = mybir.dt.bfloat16
F32 = mybir.dt.float32
F8 = mybir.dt.float8e4
DR = mybir.MatmulPerfMode.DoubleRow
W_SCALE = 64.0           # all fp8 weights scaled by this on host
O_SCALE = 8.0            # otc = O * 8 (via ones=1/8 in denominator)
G_SCALE = 8.0            # g1 partials carried x8 through the AllReduce
EBIAS = -5.0             # exp(s*SCALE + EBIAS): keeps P in fp8 range

B = 2048
HID = 4096
NH = 32
HD = 128
GH = 1024
NC_ = 8
HPC = NH // NC_          # heads per core = 4
HS = HID // NC_          # hid shard = 512
GS = GH // NC_           # gate-hidden shard = 128
SCALE = 1.0 / float(np.sqrt(HD))

KT_TILES = HID // 128    # 32 k-tiles over the 4096 contraction
KP = KT_TILES // 2       # 16 DoubleRow k-steps
JT = B // 128            # 16 j-tiles over keys
JP = JT // 2             # 8 DoubleRow j-steps
IC = B // 512            # 4 i-chunks of 512 over batch

GELU_FUNC = mybir.ActivationFunctionType.Gelu


def _build_program(allones: bool):
    nc = bacc.Bacc(
        "TRN2",
        target_bir_lowering=False,
        debug=False,
        enable_asserts=False,
        num_devices=NC_,
    )

    # ---- I/O declarations (per-core shapes) ----
    xt_d = nc.dram_tensor("xt", [128, IC, KT_TILES, 512], F8, kind="ExternalInput").ap()
    xs_d = nc.dram_tensor("xshard", [128, 4, B], F8, kind="ExternalInput").ap()
    wq_d = nc.dram_tensor("wq", [128, KT_TILES, HS], F8, kind="ExternalInput").ap()
    wk_d = nc.dram_tensor("wk", [128, KT_TILES, HS], F8, kind="ExternalInput").ap()
    wv_d = nc.dram_tensor("wv", [128, KT_TILES, HS], F8, kind="ExternalInput").ap()
    wo_d = nc.dram_tensor("wo", [128, KT_TILES, HS], F8, kind="ExternalInput").ap()
    gw1xr_d = nc.dram_tensor("gw1xr", [128, 4, GH], F8, kind="ExternalInput").ap()
    gw1c_d = nc.dram_tensor("gw1c", [128, 4, GH], F8, kind="ExternalInput").ap()
    gw2_d = nc.dram_tensor("gw2", [128, NC_, HS], F8, kind="ExternalInput").ap()
    gb1_d = nc.dram_tensor("gb1", [GS, 1], F32, kind="ExternalInput").ap()
    gb2_d = nc.dram_tensor("gb2", [128, 4], F32, kind="ExternalInput").ap()
    mask01_d = nc.dram_tensor("mask01", [128, JT], BF16, kind="ExternalInput").ap()
    diagm_d = nc.dram_tensor("diagm", [128, 128], F8, kind="ExternalInput").ap()
    out_d = nc.dram_tensor("out", [HS, B], BF16, kind="ExternalOutput").ap()

    groups = [list(range(NC_))]

    with tile.TileContext(nc) as tc:
        with (
            tc.tile_pool(name="persist", bufs=1) as persist,
            tc.tile_pool(name="psum", bufs=1, space="PSUM") as psum,
            tc.tile_pool(name="dram", bufs=1, space="DRAM") as dram,
        ):
            # ---------- persistent SBUF ----------
            qt_sb = persist.tile([128, HPC, B], BF16)     # [d, head, i] 2MB
            kt_sb = persist.tile([128, HPC, B], BF16)     # 2MB
            v_sb = persist.tile([128, JT, HS], F8)        # [j_in, j_tile, hd] 1MB
            xs_sb = persist.tile([128, 4, B], F8)         # own X^T shard 1MB
            mask01_sb = persist.tile([128, JT], BF16)
            diagm_sb = persist.tile([128, 128], F8)
            ones_sb = persist.tile([128, 2, 128], F8)
            ebias_sb = persist.tile([128, 1], F32)
            gb1_sb = persist.tile([GS, 1], F32)
            gb2_sb = persist.tile([128, 4], F32)
            # weights that live through the block phase
            wq_sb = persist.tile([128, KT_TILES, HS], F8)     # 2MB
            wo_sb = persist.tile([128, KT_TILES, HS], F8)     # 2MB
            gw1xr_sb = persist.tile([128, 4, GH], F8)
            gw1c_sb = persist.tile([128, 4, GH], F8)
            gw2_sb = persist.tile([128, NC_, HS], F8)
            cacc = persist.tile([128, 4, B], F8)

            nc.vector.memset(ones_sb, 1.0 / O_SCALE)
            nc.vector.memset(ebias_sb, EBIAS)

            # ---------- DRAM bounce buffers for collectives ----------
            ag_in = dram.tile([HPC, IC, 128, 512], F8)
            ag_out = [[None] * IC for _ in range(HPC)]
            for h in range(HPC):
                for q in range(IC):
                    ag_out[h][q] = dram.tile(
                        [NC_ * 128, 512], F8, addr_space="Shared",
                        name=f"ag_out{h}_{q}"
                    )
            rs_in_c, rs_out_c, ag2_in_c, ag2_out_c = [], [], [], []
            for icc in range(IC):
                rs_in_c.append(dram.tile([GH, 512], F8, name=f"rs_in{icc}"))
                rs_out_c.append(dram.tile([GS, 512], F8, name=f"rs_out{icc}"))
                ag2_in_c.append(dram.tile([GS, 512], F8, name=f"ag2_in{icc}"))
                ag2_out_c.append(dram.tile([GH, 512], F8, addr_space="Shared",
                                           name=f"ag2_out{icc}"))

            # warmups with the same shapes as the real collectives so the
            # first real op doesn't pay the cold-path cost
            warm_rs_i = dram.tile([GH, 512], F8)
            warm_rs_o = dram.tile([GS, 512], F8)
            warm_ag_i = dram.tile([128, 512], F8)
            warm_ag_o = dram.tile([NC_ * 128, 512], F8, addr_space="Shared")
            nc.gpsimd.collective_compute(
                "ReduceScatter", mybir.AluOpType.add, replica_groups=groups,
                ins=[warm_rs_i.opt()], outs=[warm_rs_o.opt()],
            )
            nc.gpsimd.collective_compute(
                "AllGather", mybir.AluOpType.bypass, replica_groups=groups,
                ins=[warm_ag_i.opt()], outs=[warm_ag_o.opt()],
            )

            with tc.tile_pool(name="main", bufs=1) as mp:
                # ---- streaming X quarter loader (sync queue) ----
                def load_xt(q):
                    xt_q = mp.tile([128, KT_TILES, 512], F8, tag="xt",
                                   bufs=2, name="xt_q")
                    nc.sync.dma_start(out=xt_q, in_=xt_d[:, q])
                    return xt_q

                # K/V weights in a pool released after the K/V passes
                with tc.tile_pool(name="pkv", bufs=1) as pkv:
                    wk_sb = pkv.tile([128, KT_TILES, HS], F8, tag="wk", bufs=1)
                    wv_sb = pkv.tile([128, KT_TILES, HS], F8, tag="wv", bufs=1)

                    # interleave the first X chunk with the first K-weight
                    # chunk so the first matmul starts ASAP
                    xt_first = mp.tile([128, KT_TILES, 512], F8, tag="xt",
                                       bufs=2, name="xt_q")
                    nc.sync.dma_start(out=xt_first[:, 0:8, :],
                                      in_=xt_d[:, 0, 0:8, :])
                    nc.sync.dma_start(out=wk_sb[:, 0:8, :], in_=wk_d[:, 0:8, :])
                    nc.sync.dma_start(out=xt_first[:, 8:32, :],
                                      in_=xt_d[:, 0, 8:32, :])
                    nc.sync.dma_start(out=wk_sb[:, 8:32, :],
                                      in_=wk_d[:, 8:32, :])
                    nc.sync.dma_start(out=wv_sb, in_=wv_d)
                    # remaining weights on the scalar DMA queue so they don't
                    # block the K/V-pass X streaming on the sync queue
                    nc.scalar.dma_start(out=wq_sb, in_=wq_d)
                    nc.scalar.dma_start(out=wo_sb, in_=wo_d)
                    nc.scalar.dma_start(out=xs_sb, in_=xs_d)
                    nc.scalar.dma_start(out=gw1xr_sb, in_=gw1xr_d)
                    nc.scalar.dma_start(out=gw1c_sb, in_=gw1c_d)
                    nc.scalar.dma_start(out=gw2_sb, in_=gw2_d)
                    if not allones:
                        nc.scalar.dma_start(out=mask01_sb, in_=mask01_d)
                    nc.scalar.dma_start(out=diagm_sb, in_=diagm_d)
                    nc.scalar.dma_start(out=gb1_sb, in_=gb1_d)
                    nc.scalar.dma_start(out=gb2_sb, in_=gb2_d)

                    def proj_dr(wsb, msl, xt_q, ps):
                        for k in range(KP):
                            nc.tensor.matmul(
                                ps,
                                lhsT=wsb[:, 2 * k:2 * k + 2, msl],
                                rhs=xt_q[:, 2 * k:2 * k + 2, :],
                                start=(k == 0),
                                stop=(k == KP - 1),
                                perf_mode=DR,
                            )

                    # ---- K pass ----
                    for q in range(IC):
                        isl = slice(q * 512, (q + 1) * 512)
                        xt_q = xt_first if q == 0 else load_xt(q)
                        for m in range(4):
                            ps = psum.tile([128, 512], F32, tag="mm", bufs=2,
                                           name="ps_k")
                            proj_dr(wk_sb, slice(m * 128, (m + 1) * 128),
                                    xt_q, ps)
                            nc.vector.tensor_scalar_mul(
                                kt_sb[:, m, isl], ps, 1.0 / W_SCALE
                            )

                    # ---- V pass ----
                    for q in range(IC):
                        xt_q = load_xt(q)
                        for it in range(4):
                            ps = psum.tile([128, 512], F32, tag="mm", bufs=2,
                                           name="ps_v")
                            for k in range(KP):
                                nc.tensor.matmul(
                                    ps,
                                    lhsT=xt_q[:, 2 * k:2 * k + 2,
                                              it * 128:(it + 1) * 128],
                                    rhs=wv_sb[:, 2 * k:2 * k + 2, :],
                                    start=(k == 0),
                                    stop=(k == KP - 1),
                                    perf_mode=DR,
                                )
                            nc.vector.tensor_scalar_mul(
                                v_sb[:, q * 4 + it, :], ps, 1.0 / W_SCALE
                            )

                # ======== interleaved block phase ========
                blocks = [(h, q) for q in range(IC) for h in range(HPC)]

                def qproj(m, q):
                    isl = slice(q * 512, (q + 1) * 512)
                    ps = psum.tile([128, 512], F32, tag="mm", bufs=2,
                                   name="ps_q")
                    for k in range(KP):
                        nc.tensor.matmul(
                            ps,
                            lhsT=wq_sb[:, 2 * k:2 * k + 2,
                                       m * 128:(m + 1) * 128],
                            rhs=xt_blk[q][:, 2 * k:2 * k + 2, :],
                            start=(k == 0),
                            stop=(k == KP - 1),
                            perf_mode=DR,
                        )
                    nc.vector.tensor_scalar_mul(
                        qt_sb[:, m, isl], ps, 1.0 / W_SCALE
                    )

                def attention_block(h, q):
                    qsl = slice(q * 512, (q + 1) * 512)
                    pt = mp.tile([128, JT, 512], F8, tag="pt", bufs=2,
                                 name="pt")
                    for jp in range(JP):
                        st = psum.tile([128, 2, 512], F32, tag="st",
                                       bufs=2, name="st")
                        for u in range(2):
                            j = 2 * jp + u
                            nc.tensor.matmul(
                                st[:, u, :],
                                lhsT=kt_sb[:, h, j * 128:(j + 1) * 128],
                                rhs=qt_sb[:, h, qsl],
                                start=True,
                                stop=True,
                            )
                        nc.scalar.activation(
                            pt[:, 2 * jp:2 * jp + 2, :],
                            st,
                            mybir.ActivationFunctionType.Exp,
                            bias=ebias_sb,
                            scale=SCALE,
                        )
                        for u in range(2):
                            j = 2 * jp + u
                            if not allones:
                                nc.vector.tensor_scalar_mul(
                                    pt[:, j, :], pt[:, j, :],
                                    mask01_sb[:, j:j + 1],
                                )
                            if j // 4 == q:
                                c0 = (j % 4) * 128
                                nc.vector.tensor_mul(
                                    pt[:, j, c0:c0 + 128],
                                    pt[:, j, c0:c0 + 128],
                                    diagm_sb,
                                )
                    den_ps = psum.tile([128, 512], F32, tag="acc", bufs=2,
                                       name="den_ps")
                    for jp in range(JP):
                        nc.tensor.matmul(
                            den_ps,
                            lhsT=ones_sb,
                            rhs=pt[:, 2 * jp:2 * jp + 2, :],
                            start=(jp == 0),
                            stop=(jp == JP - 1),
                            perf_mode=DR,
                        )
                    ot_ps = psum.tile([128, 512], F32, tag="acc", bufs=2,
                                      name="ot_ps")
                    for jp in range(JP):
                        nc.tensor.matmul(
                            ot_ps,
                            lhsT=v_sb[:, 2 * jp:2 * jp + 2,
                                      h * 128:(h + 1) * 128],
                            rhs=pt[:, 2 * jp:2 * jp + 2, :],
                            start=(jp == 0),
                            stop=(jp == JP - 1),
                            perf_mode=DR,
                        )
                    rec = mp.tile([128, 512], F32, tag="rec", bufs=1)
                    nc.vector.reciprocal_approx_fast(out=rec, in_=den_ps)
                    otc = mp.tile([128, 512], F8, tag="otc", bufs=2)
                    nc.vector.tensor_mul(otc, ot_ps, rec)
                    nc.sync.dma_start(out=ag_in[h, q], in_=otc)
                    nc.gpsimd.collective_compute(
                        "AllGather",
                        mybir.AluOpType.bypass,
                        replica_groups=groups,
                        ins=[ag_in[h, q].opt()],
                        outs=[ag_out[h][q].opt()],
                    )

                def otg_load(t, ic):
                    otg = mp.tile([128, NC_, 512], F8, tag="otg", bufs=2,
                                  name="otg")
                    nc.sync.dma_start(
                        out=otg,
                        in_=ag_out[t][ic].rearrange("(r p) i -> p r i", p=128),
                    )
                    return otg

                def outproj_group(t, ic, otg):
                    csl = slice(ic * 512, (ic + 1) * 512)
                    for m in range(4):
                        ps = psum.tile([128, 512], F32, tag="mm", bufs=2,
                                       name="ps_wo")
                        for r in range(NC_ // 2):
                            nc.tensor.matmul(
                                ps,
                                lhsT=wo_sb[:, t * NC_ + 2 * r:
                                           t * NC_ + 2 * r + 2,
                                           m * 128:(m + 1) * 128],
                                rhs=otg[:, 2 * r:2 * r + 2, :],
                                start=(r == 0),
                                stop=(r == NC_ // 2 - 1),
                                perf_mode=DR,
                            )
                        if t == 0:
                            nc.vector.tensor_scalar_mul(
                                cacc[:, m, csl], ps, 1.0 / (W_SCALE * O_SCALE)
                            )
                        else:
                            nc.vector.scalar_tensor_tensor(
                                cacc[:, m, csl], ps, 1.0 / (W_SCALE * O_SCALE),
                                cacc[:, m, csl],
                                op0=mybir.AluOpType.mult,
                                op1=mybir.AluOpType.add,
                            )

                def g1c_rs(ic):
                    # gate-W1 partial over this core's 1024-row contraction
                    # shard ([own X^T shard rows; own cross^T shard]), all gh
                    # columns; fp8 ReduceScatter sums across cores.
                    csl = slice(ic * 512, (ic + 1) * 512)
                    for gm in range(NC_):
                        gmsl = slice(gm * 128, (gm + 1) * 128)
                        ps = psum.tile([128, 512], F32, tag="mm", bufs=2,
                                       name="ps_g1c")
                        for r in range(2):
                            nc.tensor.matmul(
                                ps,
                                lhsT=gw1xr_sb[:, 2 * r:2 * r + 2, gmsl],
                                rhs=xs_sb[:, 2 * r:2 * r + 2, csl],
                                start=(r == 0),
                                stop=False,
                                perf_mode=DR,
                            )
                        for r in range(2):
                            nc.tensor.matmul(
                                ps,
                                lhsT=gw1c_sb[:, 2 * r:2 * r + 2, gmsl],
                                rhs=cacc[:, 2 * r:2 * r + 2, csl],
                                start=False,
                                stop=(r == 1),
                                perf_mode=DR,
                            )
                        g1c_ch = mp.tile([128, 512], F8, tag="g1cch", bufs=2)
                        nc.vector.tensor_scalar_mul(
                            g1c_ch, ps, G_SCALE / W_SCALE
                        )
                        nc.sync.dma_start(
                            out=rs_in_c[ic][gm * 128:(gm + 1) * 128, :],
                            in_=g1c_ch,
                        )
                    nc.gpsimd.collective_compute(
                        "ReduceScatter",
                        mybir.AluOpType.add,
                        replica_groups=groups,
                        ins=[rs_in_c[ic].opt()],
                        outs=[rs_out_c[ic].opt()],
                    )

                def gelu_ag(ic):
                    # gelu on this core's gh-shard of the summed g1, then
                    # AllGather the activated shard
                    rsum = mp.tile([128, 512], F8, tag="rsum", bufs=1)
                    nc.sync.dma_start(out=rsum, in_=rs_out_c[ic])
                    gt_ch = mp.tile([128, 512], F8, tag="gt", bufs=1)
                    nc.scalar.activation(gt_ch, rsum, GELU_FUNC,
                                         bias=gb1_sb, scale=1.0 / G_SCALE)
                    nc.sync.dma_start(out=ag2_in_c[ic], in_=gt_ch)
                    nc.gpsimd.collective_compute(
                        "AllGather",
                        mybir.AluOpType.bypass,
                        replica_groups=groups,
                        ins=[ag2_in_c[ic].opt()],
                        outs=[ag2_out_c[ic].opt()],
                    )

                def gtf_load(ic):
                    gtf = mp.tile([128, NC_, 512], F8, tag="gtf",
                                  bufs=1, name="gtf")
                    nc.sync.dma_start(
                        out=gtf,
                        in_=ag2_out_c[ic].rearrange("(r p) i -> p r i", p=128),
                    )
                    return gtf

                def gate_chain(ic, gtf):
                    csl = slice(ic * 512, (ic + 1) * 512)
                    for m in range(4):
                        ps = psum.tile([128, 512], F32, tag="mm", bufs=2,
                                       name="ps_gw2")
                        for r in range(NC_ // 2):
                            nc.tensor.matmul(
                                ps,
                                lhsT=gw2_sb[:, 2 * r:2 * r + 2,
                                            m * 128:(m + 1) * 128],
                                rhs=gtf[:, 2 * r:2 * r + 2, :],
                                start=(r == 0),
                                stop=(r == NC_ // 2 - 1),
                                perf_mode=DR,
                            )
                        gate_ch = mp.tile([128, 512], BF16, tag="gate",
                                          bufs=2)
                        nc.scalar.activation(
                            gate_ch, ps,
                            mybir.ActivationFunctionType.Sigmoid,
                            bias=gb2_sb[:, m:m + 1], scale=1.0 / W_SCALE,
                        )
                        outt = mp.tile([128, 512], BF16, tag="outt", bufs=2)
                        nc.vector.tensor_mul(outt, gate_ch, cacc[:, m, csl])
                        nc.sync.dma_start(
                            out=out_d[m * 128:(m + 1) * 128, csl], in_=outt
                        )

                # X quarters for the Q projections (re-streamed)
                xt_blk = {}
                xt_blk[0] = load_xt(0)
                for m in range(4):
                    qproj(m, 0)

                otg_pend = {}
                gtf_pend = {}
                for s, (h, q) in enumerate(blocks):
                    if h == 0 and q + 1 < IC:
                        xt_blk[q + 1] = load_xt(q + 1)
                    if s >= 2:
                        # prefetch the out_proj operand consumed next block
                        t2, ic2 = blocks[s - 2]
                        otg_pend[(t2, ic2)] = otg_load(t2, ic2)
                    attention_block(h, q)
                    if s >= 3:
                        t3, ic3 = blocks[s - 3]
                        outproj_group(t3, ic3, otg_pend.pop((t3, ic3)))
                        if t3 == HPC - 1:
                            g1c_rs(ic3)
                    if q + 1 < IC:
                        qproj(h, q + 1)
                    # gate chain steps at fixed offsets after each chunk's
                    # ReduceScatter (RS for chunk ic triggers at block 4ic+6)
                    for icg in range(2):
                        if s == 4 * icg + 8:
                            gelu_ag(icg)
                        if s == 4 * icg + 10:
                            gtf_pend[icg] = gtf_load(icg)
                        if s == 4 * icg + 11:
                            gate_chain(icg, gtf_pend.pop(icg))

                # ---- tail: last three out_proj groups, chunk 2+3 chains ----
                t3, ic3 = blocks[-3]
                outproj_group(t3, ic3, otg_pend.pop((t3, ic3)))
                for s2 in (len(blocks) - 2, len(blocks) - 1):
                    t2, ic2 = blocks[s2]
                    otg = otg_load(t2, ic2)
                    outproj_group(t2, ic2, otg)
                    if t2 == HPC - 1:
                        g1c_rs(ic2)
                # chunk 2 chain fills the RS(3) latency
                gelu_ag(IC - 2)
                gtf2 = gtf_load(IC - 2)
                gate_chain(IC - 2, gtf2)
                gelu_ag(IC - 1)
                gtf3 = gtf_load(IC - 1)
                gate_chain(IC - 1, gtf3)

    nc.compile()
    return nc


def _q8(x, scale=1.0):
    f8 = ml_dtypes.float8_e4m3
    return np.ascontiguousarray(
        np.clip(np.asarray(x, dtype=np.float32) * scale, -240.0, 240.0)
    ).astype(f8)


def _make_in_maps(inputs):
    f32 = np.float32
    X = np.asarray(inputs["hidden_states"], dtype=f32)
    mask = np.asarray(inputs["attention_mask"])
    Wq = np.asarray(inputs["Wq"], dtype=f32)
    Wk = np.asarray(inputs["Wk"], dtype=f32)
    Wv = np.asarray(inputs["Wv"], dtype=f32)
    Wo = np.asarray(inputs["Wo"], dtype=f32)
    gW1 = np.asarray(inputs["gW1"], dtype=f32)
    gb1 = np.asarray(inputs["gb1"], dtype=f32)
    gW2 = np.asarray(inputs["gW2"], dtype=f32)
    gb2 = np.asarray(inputs["gb2"], dtype=f32)

    XT8 = _q8(X.T)                                       # [4096, 2048]
    # pre-tile to [partition, quarter, k-tile, 512] so every DMA moves
    # large contiguous per-partition segments (128 x 16KB descriptors)
    XTT = np.ascontiguousarray(
        XT8.reshape(KT_TILES, 128, IC, 512).transpose(1, 2, 0, 3))

    def _tile_w(w8):  # [K, M] -> [128, K/128, M]
        kt = w8.shape[0] // 128
        return np.ascontiguousarray(
            w8.reshape(kt, 128, w8.shape[1]).transpose(1, 0, 2))
    # Wo row permutation to match per-head AllGather chunk assembly:
    # OT_full row (t*1024 + r*128 + d) holds global head (4r+t), dim d.
    perm = np.empty(HID, dtype=np.int64)
    for t in range(HPC):
        for r in range(NC_):
            g = 4 * r + t
            perm[t * 1024 + r * 128:t * 1024 + (r + 1) * 128] = np.arange(
                g * 128, (g + 1) * 128
            )
    Wo_p = Wo[perm]
    mask01_t = np.ascontiguousarray(
        mask.astype(f32).reshape(JT, 128).T).astype(ml_dtypes.bfloat16)
    diagm = _q8(1.0 - np.eye(128, dtype=f32))

    in_maps = []
    for c in range(NC_):
        hsl = slice(c * HS, (c + 1) * HS)
        gsl = slice(c * GS, (c + 1) * GS)
        in_maps.append({
            "xt": XTT,
            "xshard": _tile_w(XT8[hsl]),
            "wq": _tile_w(_q8(Wq[:, hsl], W_SCALE)),
            "wk": _tile_w(_q8(Wk[:, hsl], W_SCALE)),
            "wv": _tile_w(_q8(Wv[:, hsl], W_SCALE)),
            "wo": _tile_w(_q8(Wo_p[:, hsl], W_SCALE)),
            "gw1xr": _tile_w(_q8(gW1[c * HS:(c + 1) * HS], W_SCALE)),
            "gw1c": _tile_w(_q8(gW1[HID + c * HS:HID + (c + 1) * HS], W_SCALE)),
            "gw2": _tile_w(_q8(gW2[:, hsl], W_SCALE)),
            "gb1": np.ascontiguousarray(gb1[gsl].reshape(GS, 1)),
            "gb2": np.ascontiguousarray(gb2[hsl].reshape(4, 128).T),
            "mask01": mask01_t,
            "diagm": diagm,
        })
    return in_maps


_NC_CACHE = {}


def _run(inputs, trace=False):
    allones = bool(np.asarray(inputs["attention_mask"]).all())
    nc = _NC_CACHE.get(allones)
    if nc is None:
        nc = _build_program(allones)
        _NC_CACHE[allones] = nc
    in_maps = _make_in_maps(inputs)
    res = bass_utils.run_bass_kernel_spmd(
        nc, in_maps, core_ids=list(range(NC_)), trace=trace
    )
    shards = [np.asarray(res.results[c]["out"], dtype=np.float32)
              for c in range(NC_)]
    gated = np.concatenate(shards, axis=0).T  # gate * cross, [2048, 4096]
    out = np.asarray(inputs["hidden_states"], dtype=np.float32) + gated
    return np.ascontiguousarray(out), res


def kernel(**inputs) -> np.ndarray:
    out, _ = _run(inputs, trace=False)
    return out



# revision 8
# speedup vs baseline: 1.0119x; 1.0119x over previous
"""CrossBatchAttention Trainium2 kernel — 8-core tensor-parallel SPMD.

v3: same numerics as v2 (fp8 DoubleRow everywhere), restructured schedule:

  - Merged K/V/Q projection pass: one streaming sweep over the 4 X^T
    quarters computes kt, v and qt together (X loaded once, not 3x).
    The sync DMA queue is free of X traffic during the block phase.
  - Fine-grained startup: quarter-0 X and Wk arrive in 4-k-tile chunks
    consumed by a k-outer loop over 4 live PSUM banks, so the first
    matmul issues as soon as the first 256KB lands.
  - Paired OT AllGathers: heads {0,1} and {2,3} of each batch quarter
    share one AllGather (128KB in / 1MB out) -> 8 collectives instead
    of 16, halving CC-core occupancy.
  - Tight tail: outproj lags its AG by 2 blocks, chunk-3's g1c/RS is
    issued right after the last outproj, and collective triggers are
    ordered so the CC FIFO never parks a ready collective behind an
    unready one.

Quantization (unchanged from v2): X fp8, W* fp8 x64 (/64 on PSUM exit),
qt/kt bf16, P = exp(s/sqrt(d) - 5.0) fp8, ones = 1/8 so rec = 8/den,
otc = O*rec fp8, cacc fp8, g1 partials fp8 x8 through the RS,
sigmoid(logits/64). Host: concat 8 [512,2048] bf16 shards, transpose,
add X -> f32.
"""

import numpy as np
import ml_dtypes

import concourse.bass as bass
import concourse.mybir as mybir
import concourse.tile as tile
from concourse import bacc
from concourse import bass_utils

BF16 = mybir.dt.bfloat16
F32 = mybir.dt.float32
F8 = mybir.dt.float8e4
DR = mybir.MatmulPerfMode.DoubleRow
W_SCALE = 64.0           # all fp8 weights scaled by this on host
O_SCALE = 8.0            # otc = O * 8 (via ones=1/8 in denominator)
G_SCALE = 8.0            # g1 partials carried x8 through the RS
EBIAS = -5.0             # exp(s*SCALE + EBIAS): keeps P in fp8 range

B = 2048
HID = 4096
NH = 32
HD = 128
GH = 1024
NC_ = 8
HPC = NH // NC_          # heads per core = 4
HS = HID // NC_          # hid shard = 512
GS = GH // NC_           # gate-hidden shard = 128
SCALE = 1.0 / float(np.sqrt(HD))

KT_TILES = HID // 128    # 32 k-tiles over the 4096 contraction
KP = KT_TILES // 2       # 16 DoubleRow k-steps
JT = B // 128            # 16 j-tiles over keys
JP = JT // 2             # 8 DoubleRow j-steps
IC = B // 512            # 4 i-chunks of 512 over batch

GELU_FUNC = mybir.ActivationFunctionType.Gelu


def _build_program(allones: bool):
    nc = bacc.Bacc(
        "TRN2",
        target_bir_lowering=False,
        debug=False,
        enable_asserts=False,
        num_devices=NC_,
    )

    # ---- I/O declarations (per-core shapes) ----
    xt_d = nc.dram_tensor("xt", [128, IC, KT_TILES, 512], F8, kind="ExternalInput").ap()
    xs_d = nc.dram_tensor("xshard", [128, 4, B], F8, kind="ExternalInput").ap()
    wq_d = nc.dram_tensor("wq", [128, KT_TILES, HS], F8, kind="ExternalInput").ap()
    wk_d = nc.dram_tensor("wk", [128, KT_TILES, HS], F8, kind="ExternalInput").ap()
    wv_d = nc.dram_tensor("wv", [128, KT_TILES, HS], F8, kind="ExternalInput").ap()
    wo_d = nc.dram_tensor("wo", [128, KT_TILES, HS], F8, kind="ExternalInput").ap()
    gw1xr_d = nc.dram_tensor("gw1xr", [128, 4, GH], F8, kind="ExternalInput").ap()
    gw1c_d = nc.dram_tensor("gw1c", [128, 4, GH], F8, kind="ExternalInput").ap()
    gw2_d = nc.dram_tensor("gw2", [128, NC_, HS], F8, kind="ExternalInput").ap()
    gb1_d = nc.dram_tensor("gb1", [GS, 1], F32, kind="ExternalInput").ap()
    gb2_d = nc.dram_tensor("gb2", [128, 4], F32, kind="ExternalInput").ap()
    mask01_d = nc.dram_tensor("mask01", [128, JT], BF16, kind="ExternalInput").ap()
    diagm_d = nc.dram_tensor("diagm", [128, 128], F8, kind="ExternalInput").ap()
    out_d = nc.dram_tensor("out", [HS, B], BF16, kind="ExternalOutput").ap()

    groups = [list(range(NC_))]

    with tile.TileContext(nc) as tc:
        with (
            tc.tile_pool(name="persist", bufs=1) as persist,
            tc.tile_pool(name="psum", bufs=1, space="PSUM") as psum,
            tc.tile_pool(name="dram", bufs=1, space="DRAM") as dram,
        ):
            # ---------- persistent SBUF ----------
            qt_sb = persist.tile([128, HPC, B], BF16)     # [d, head, i] 2MB
            kt_sb = persist.tile([128, HPC, B], BF16)     # 2MB
            v_sb = persist.tile([128, JT, HS], F8)        # [j_in, j_tile, hd] 1MB
            xs_sb = persist.tile([128, 4, B], F8)         # own X^T shard 1MB
            mask01_sb = persist.tile([128, JT], BF16)
            diagm_sb = persist.tile([128, 128], F8)
            ones_sb = persist.tile([128, 2, 128], F8)
            ebias_sb = persist.tile([128, 1], F32)
            gb1_sb = persist.tile([GS, 1], F32)
            gb2_sb = persist.tile([128, 4], F32)
            # weights that live through the block phase
            wo_sb = persist.tile([128, KT_TILES, HS], F8)     # 2MB
            gw1xr_sb = persist.tile([128, 4, GH], F8)
            gw1c_sb = persist.tile([128, 4, GH], F8)
            gw2_sb = persist.tile([128, NC_, HS], F8)
            cacc = persist.tile([128, 4, B], F8)

            nc.vector.memset(ones_sb, 1.0 / O_SCALE)
            nc.vector.memset(ebias_sb, EBIAS)

            # ---------- DRAM bounce buffers for collectives ----------
            # paired OT AllGather: rank buffer [2, 128, 512] (heads 2p,2p+1)
            ag_in = [[None] * IC for _ in range(2)]
            ag_out = [[None] * IC for _ in range(2)]
            for p in range(2):
                for q in range(IC):
                    ag_in[p][q] = dram.tile([2, 128, 512], F8,
                                            name=f"ag_in{p}_{q}")
                    ag_out[p][q] = dram.tile(
                        [NC_ * 256, 512], F8, addr_space="Shared",
                        name=f"ag_out{p}_{q}"
                    )
            rs_in_c, rs_out_c, ag2_in_c, ag2_out_c = [], [], [], []
            for icc in range(IC):
                rs_in_c.append(dram.tile([GH, 512], F8, name=f"rs_in{icc}"))
                rs_out_c.append(dram.tile([GS, 512], F8, name=f"rs_out{icc}"))
                ag2_in_c.append(dram.tile([GS, 512], F8, name=f"ag2_in{icc}"))
                ag2_out_c.append(dram.tile([GH, 512], F8, addr_space="Shared",
                                           name=f"ag2_out{icc}"))

            # warmups with the same shapes as the real collectives so the
            # first real op doesn't pay the cold-path cost
            warm_rs_i = dram.tile([GH, 512], F8)
            warm_rs_o = dram.tile([GS, 512], F8)
            warm_ag_i = dram.tile([2, 128, 512], F8)
            warm_ag_o = dram.tile([NC_ * 256, 512], F8, addr_space="Shared")
            warm_ag2_i = dram.tile([GS, 512], F8)
            warm_ag2_o = dram.tile([GH, 512], F8, addr_space="Shared")
            nc.gpsimd.collective_compute(
                "ReduceScatter", mybir.AluOpType.add, replica_groups=groups,
                ins=[warm_rs_i.opt()], outs=[warm_rs_o.opt()],
            )
            nc.gpsimd.collective_compute(
                "AllGather", mybir.AluOpType.bypass, replica_groups=groups,
                ins=[warm_ag_i.opt()], outs=[warm_ag_o.opt()],
            )
            nc.gpsimd.collective_compute(
                "AllGather", mybir.AluOpType.bypass, replica_groups=groups,
                ins=[warm_ag2_i.opt()], outs=[warm_ag2_o.opt()],
            )

            with tc.tile_pool(name="main", bufs=1) as mp:
                # ======== merged K/V/Q projection pass ========
                with tc.tile_pool(name="pkvq", bufs=1) as pkvq:
                    wk_sb = pkvq.tile([128, KT_TILES, HS], F8, tag="wk", bufs=1)
                    wv_sb = pkvq.tile([128, KT_TILES, HS], F8, tag="wv", bufs=1)
                    wq_sb = pkvq.tile([128, KT_TILES, HS], F8, tag="wq", bufs=1)

                    # quarter-0 X and Wk in 4-k-tile chunks (256KB each),
                    # interleaved so the k-outer loop starts ASAP
                    xt_first = pkvq.tile([128, KT_TILES, 512], F8, tag="xt",
                                         bufs=2, name="xt_q")
                    NCH = 8   # chunks of 4 k-tiles
                    for ch in range(NCH):
                        ksl = slice(ch * 4, (ch + 1) * 4)
                        nc.sync.dma_start(out=xt_first[:, ksl, :],
                                          in_=xt_d[:, 0, ksl, :])
                        nc.sync.dma_start(out=wk_sb[:, ksl, :],
                                          in_=wk_d[:, ksl, :])
                    nc.sync.dma_start(out=wv_sb, in_=wv_d)

                    def load_xt(q):
                        xt_q = pkvq.tile([128, KT_TILES, 512], F8, tag="xt",
                                         bufs=2, name="xt_q")
                        nc.sync.dma_start(out=xt_q, in_=xt_d[:, q])
                        return xt_q

                    xt_next = load_xt(1)

                    # remaining weights on the scalar DMA queue so they don't
                    # block the projection-pass X streaming on the sync queue
                    nc.scalar.dma_start(out=wq_sb, in_=wq_d)
                    nc.scalar.dma_start(out=wo_sb, in_=wo_d)
                    nc.scalar.dma_start(out=xs_sb, in_=xs_d)
                    nc.scalar.dma_start(out=gw1xr_sb, in_=gw1xr_d)
                    nc.scalar.dma_start(out=gw1c_sb, in_=gw1c_d)
                    nc.scalar.dma_start(out=gw2_sb, in_=gw2_d)
                    if not allones:
                        nc.scalar.dma_start(out=mask01_sb, in_=mask01_d)
                    nc.scalar.dma_start(out=diagm_sb, in_=diagm_d)
                    nc.scalar.dma_start(out=gb1_sb, in_=gb1_d)
                    nc.scalar.dma_start(out=gb2_sb, in_=gb2_d)

                    def proj_dr(wsb, msl, xt_q, ps):
                        for k in range(KP):
                            nc.tensor.matmul(
                                ps,
                                lhsT=wsb[:, 2 * k:2 * k + 2, msl],
                                rhs=xt_q[:, 2 * k:2 * k + 2, :],
                                start=(k == 0),
                                stop=(k == KP - 1),
                                perf_mode=DR,
                            )

                    def v_pass(q, xt_q):
                        for it in range(4):
                            ps = psum.tile([128, 512], F32, tag="mm", bufs=2,
                                           name="ps_v")
                            for k in range(KP):
                                nc.tensor.matmul(
                                    ps,
                                    lhsT=xt_q[:, 2 * k:2 * k + 2,
                                              it * 128:(it + 1) * 128],
                                    rhs=wv_sb[:, 2 * k:2 * k + 2, :],
                                    start=(k == 0),
                                    stop=(k == KP - 1),
                                    perf_mode=DR,
                                )
                            nc.vector.tensor_scalar_mul(
                                v_sb[:, q * 4 + it, :], ps, 1.0 / W_SCALE
                            )

                    def q_pass(q, xt_q):
                        isl = slice(q * 512, (q + 1) * 512)
                        for m in range(4):
                            ps = psum.tile([128, 512], F32, tag="mm", bufs=2,
                                           name="ps_q")
                            proj_dr(wq_sb, slice(m * 128, (m + 1) * 128),
                                    xt_q, ps)
                            nc.vector.tensor_scalar_mul(
                                qt_sb[:, m, isl], ps, 1.0 / W_SCALE
                            )

                    # --- quarter 0: k-outer K pass over 4 live PSUM banks ---
                    # (borrow the scores' "st" tag banks; they're idle here)
                    kpsA = psum.tile([128, 2, 512], F32, tag="st", bufs=2,
                                     name="kpsA")
                    kpsB = psum.tile([128, 2, 512], F32, tag="st", bufs=2,
                                     name="kpsB")
                    kps = [kpsA[:, 0, :], kpsA[:, 1, :],
                           kpsB[:, 0, :], kpsB[:, 1, :]]
                    for ch in range(NCH):
                        for m in range(4):
                            for u in range(2):
                                k = ch * 2 + u
                                nc.tensor.matmul(
                                    kps[m],
                                    lhsT=wk_sb[:, 4 * ch + 2 * u:
                                               4 * ch + 2 * u + 2,
                                               m * 128:(m + 1) * 128],
                                    rhs=xt_first[:, 4 * ch + 2 * u:
                                                 4 * ch + 2 * u + 2, :],
                                    start=(ch == 0 and u == 0),
                                    stop=(ch == NCH - 1 and u == 1),
                                    perf_mode=DR,
                                )
                    for m in range(4):
                        nc.vector.tensor_scalar_mul(
                            kt_sb[:, m, 0:512], kps[m], 1.0 / W_SCALE
                        )
                    v_pass(0, xt_first)
                    q_pass(0, xt_first)

                    # --- quarters 1..3: standard m-outer loops ---
                    for q in range(1, IC):
                        xt_q = xt_next
                        if q + 1 < IC:
                            xt_next = load_xt(q + 1)
                        isl = slice(q * 512, (q + 1) * 512)
                        for m in range(4):
                            ps = psum.tile([128, 512], F32, tag="mm", bufs=2,
                                           name="ps_k")
                            proj_dr(wk_sb, slice(m * 128, (m + 1) * 128),
                                    xt_q, ps)
                            nc.vector.tensor_scalar_mul(
                                kt_sb[:, m, isl], ps, 1.0 / W_SCALE
                            )
                        v_pass(q, xt_q)
                        q_pass(q, xt_q)

                # ======== interleaved block phase ========
                # block s: (h, q) = (s % 4, s // 4)

                def attention_block(h, q):
                    p = h // 2
                    u = h % 2
                    qsl = slice(q * 512, (q + 1) * 512)
                    pt = mp.tile([128, JT, 512], F8, tag="pt", bufs=2,
                                 name="pt")
                    for jp in range(JP):
                        st = psum.tile([128, 2, 512], F32, tag="st",
                                       bufs=2, name="st")
                        for uu in range(2):
                            j = 2 * jp + uu
                            nc.tensor.matmul(
                                st[:, uu, :],
                                lhsT=kt_sb[:, h, j * 128:(j + 1) * 128],
                                rhs=qt_sb[:, h, qsl],
                                start=True,
                                stop=True,
                            )
                        nc.scalar.activation(
                            pt[:, 2 * jp:2 * jp + 2, :],
                            st,
                            mybir.ActivationFunctionType.Exp,
                            bias=ebias_sb,
                            scale=SCALE,
                        )
                        for uu in range(2):
                            j = 2 * jp + uu
                            if not allones:
                                nc.vector.tensor_scalar_mul(
                                    pt[:, j, :], pt[:, j, :],
                                    mask01_sb[:, j:j + 1],
                                )
                            if j // 4 == q:
                                c0 = (j % 4) * 128
                                nc.vector.tensor_mul(
                                    pt[:, j, c0:c0 + 128],
                                    pt[:, j, c0:c0 + 128],
                                    diagm_sb,
                                )
                    den_ps = psum.tile([128, 512], F32, tag="acc", bufs=2,
                                       name="den_ps")
                    for jp in range(JP):
                        nc.tensor.matmul(
                            den_ps,
                            lhsT=ones_sb,
                            rhs=pt[:, 2 * jp:2 * jp + 2, :],
                            start=(jp == 0),
                            stop=(jp == JP - 1),
                            perf_mode=DR,
                        )
                    ot_ps = psum.tile([128, 512], F32, tag="acc", bufs=2,
                                      name="ot_ps")
                    for jp in range(JP):
                        nc.tensor.matmul(
                            ot_ps,
                            lhsT=v_sb[:, 2 * jp:2 * jp + 2,
                                      h * 128:(h + 1) * 128],
                            rhs=pt[:, 2 * jp:2 * jp + 2, :],
                            start=(jp == 0),
                            stop=(jp == JP - 1),
                            perf_mode=DR,
                        )
                    rec = mp.tile([128, 512], F32, tag="rec", bufs=1)
                    nc.vector.reciprocal_approx_fast(out=rec, in_=den_ps)
                    otc = mp.tile([128, 512], F8, tag="otc", bufs=1)
                    nc.vector.tensor_mul(otc, ot_ps, rec)
                    nc.sync.dma_start(out=ag_in[p][q][u], in_=otc)
                    if u == 1:
                        nc.gpsimd.collective_compute(
                            "AllGather",
                            mybir.AluOpType.bypass,
                            replica_groups=groups,
                            ins=[ag_in[p][q].opt()],
                            outs=[ag_out[p][q].opt()],
                        )

                def otg_load(p, ic):
                    otg = mp.tile([128, 2 * NC_, 512], F8, tag="otg", bufs=2,
                                  name="otg")
                    nc.sync.dma_start(
                        out=otg,
                        in_=ag_out[p][ic].rearrange("(g j) i -> j g i", j=128),
                    )
                    return otg

                def outproj_pair(p, ic, otg):
                    # contraction over the 16 gathered head-tiles of pair p
                    csl = slice(ic * 512, (ic + 1) * 512)
                    for m in range(4):
                        ps = psum.tile([128, 512], F32, tag="mm", bufs=2,
                                       name="ps_wo")
                        for r in range(NC_):
                            nc.tensor.matmul(
                                ps,
                                lhsT=wo_sb[:, p * 16 + 2 * r:
                                           p * 16 + 2 * r + 2,
                                           m * 128:(m + 1) * 128],
                                rhs=otg[:, 2 * r:2 * r + 2, :],
                                start=(r == 0),
                                stop=(r == NC_ - 1),
                                perf_mode=DR,
                            )
                        if p == 0:
                            nc.vector.tensor_scalar_mul(
                                cacc[:, m, csl], ps, 1.0 / (W_SCALE * O_SCALE)
                            )
                        else:
                            nc.vector.scalar_tensor_tensor(
                                cacc[:, m, csl], ps, 1.0 / (W_SCALE * O_SCALE),
                                cacc[:, m, csl],
                                op0=mybir.AluOpType.mult,
                                op1=mybir.AluOpType.add,
                            )

                def g1c_rs(ic):
                    # gate-W1 partial over this core's 1024-row contraction
                    # shard ([own X^T shard rows; own cross^T shard]), all gh
                    # columns; fp8 ReduceScatter sums across cores.
                    csl = slice(ic * 512, (ic + 1) * 512)
                    for gm in range(NC_):
                        gmsl = slice(gm * 128, (gm + 1) * 128)
                        ps = psum.tile([128, 512], F32, tag="mm", bufs=2,
                                       name="ps_g1c")
                        for r in range(2):
                            nc.tensor.matmul(
                                ps,
                                lhsT=gw1xr_sb[:, 2 * r:2 * r + 2, gmsl],
                                rhs=xs_sb[:, 2 * r:2 * r + 2, csl],
                                start=(r == 0),
                                stop=False,
                                perf_mode=DR,
                            )
                        for r in range(2):
                            nc.tensor.matmul(
                                ps,
                                lhsT=gw1c_sb[:, 2 * r:2 * r + 2, gmsl],
                                rhs=cacc[:, 2 * r:2 * r + 2, csl],
                                start=False,
                                stop=(r == 1),
                                perf_mode=DR,
                            )
                        g1c_ch = mp.tile([128, 512], F8, tag="g1cch", bufs=1)
                        nc.vector.tensor_scalar_mul(
                            g1c_ch, ps, G_SCALE / W_SCALE
                        )
                        nc.sync.dma_start(
                            out=rs_in_c[ic][gm * 128:(gm + 1) * 128, :],
                            in_=g1c_ch,
                        )
                    nc.gpsimd.collective_compute(
                        "ReduceScatter",
                        mybir.AluOpType.add,
                        replica_groups=groups,
                        ins=[rs_in_c[ic].opt()],
                        outs=[rs_out_c[ic].opt()],
                    )

                def gelu_ag(ic):
                    # gelu on this core's gh-shard of the summed g1, then
                    # AllGather the activated shard
                    rsum = mp.tile([128, 512], F8, tag="rsum", bufs=1)
                    nc.sync.dma_start(out=rsum, in_=rs_out_c[ic])
                    gt_ch = mp.tile([128, 512], F8, tag="gt", bufs=1)
                    nc.scalar.activation(gt_ch, rsum, GELU_FUNC,
                                         bias=gb1_sb, scale=1.0 / G_SCALE)
                    nc.sync.dma_start(out=ag2_in_c[ic], in_=gt_ch)
                    nc.gpsimd.collective_compute(
                        "AllGather",
                        mybir.AluOpType.bypass,
                        replica_groups=groups,
                        ins=[ag2_in_c[ic].opt()],
                        outs=[ag2_out_c[ic].opt()],
                    )

                def gtf_load(ic):
                    gtf = mp.tile([128, NC_, 512], F8, tag="gtf",
                                  bufs=1, name="gtf")
                    nc.sync.dma_start(
                        out=gtf,
                        in_=ag2_out_c[ic].rearrange("(r p) i -> p r i", p=128),
                    )
                    return gtf

                def gate_chain(ic, gtf):
                    csl = slice(ic * 512, (ic + 1) * 512)
                    for m in range(4):
                        ps = psum.tile([128, 512], F32, tag="mm", bufs=2,
                                       name="ps_gw2")
                        for r in range(NC_ // 2):
                            nc.tensor.matmul(
                                ps,
                                lhsT=gw2_sb[:, 2 * r:2 * r + 2,
                                            m * 128:(m + 1) * 128],
                                rhs=gtf[:, 2 * r:2 * r + 2, :],
                                start=(r == 0),
                                stop=(r == NC_ // 2 - 1),
                                perf_mode=DR,
                            )
                        gate_ch = mp.tile([128, 512], BF16, tag="gate",
                                          bufs=2)
                        nc.scalar.activation(
                            gate_ch, ps,
                            mybir.ActivationFunctionType.Sigmoid,
                            bias=gb2_sb[:, m:m + 1], scale=1.0 / W_SCALE,
                        )
                        outt = mp.tile([128, 512], BF16, tag="outt", bufs=1)
                        nc.vector.tensor_mul(outt, gate_ch, cacc[:, m, csl])
                        nc.sync.dma_start(
                            out=out_d[m * 128:(m + 1) * 128, csl], in_=outt
                        )

                # schedule:
                #   otg_A(ic) @ 4ic+2   outproj_A(ic) @ 4ic+3
                #   otg_B(ic) @ 4ic+4   outproj_B+g1c_rs(ic) @ 4ic+5
                #   gelu_ag(ic) @ 4ic+7  gtf(ic) @ 4ic+8  gate(ic) @ 4ic+9
                otg_pend = {}
                gtf_pend = {}
                for s in range(16):
                    h, q = s % 4, s // 4
                    attention_block(h, q)
                    r4 = s % 4
                    if r4 == 2:
                        otg_pend[(0, q)] = otg_load(0, q)
                    if r4 == 0 and s >= 4:
                        otg_pend[(1, q - 1)] = otg_load(1, q - 1)
                    if r4 == 3:
                        outproj_pair(0, q, otg_pend.pop((0, q)))
                    if r4 == 1 and s >= 5:
                        outproj_pair(1, q - 1, otg_pend.pop((1, q - 1)))
                        g1c_rs(q - 1)
                    if r4 == 3 and s >= 7:
                        gelu_ag((s - 7) // 4)
                    if r4 == 0 and s >= 8:
                        gtf_pend[(s - 8) // 4] = gtf_load((s - 8) // 4)
                    if r4 == 1 and s >= 9:
                        ic = (s - 9) // 4
                        gate_chain(ic, gtf_pend.pop(ic))

                # ---- tail: chunk 3 pair-B chain + chunk 2/3 gate chains ----
                otg_b3 = otg_load(1, 3)
                outproj_pair(1, 3, otg_b3)
                g1c_rs(3)
                gtf_pend[2] = gtf_load(2)
                gate_chain(2, gtf_pend.pop(2))
                gelu_ag(3)
                gtf_pend[3] = gtf_load(3)
                gate_chain(3, gtf_pend.pop(3))

    nc.compile()
    return nc


def _q8(x, scale=1.0):
    f8 = ml_dtypes.float8_e4m3
    return np.ascontiguousarray(
        np.clip(np.asarray(x, dtype=np.float32) * scale, -240.0, 240.0)
    ).astype(f8)


def _make_in_maps(inputs):
    f32 = np.float32
    X = np.asarray(inputs["hidden_states"], dtype=f32)
    mask = np.asarray(inputs["attention_mask"])
    Wq = np.asarray(inputs["Wq"], dtype=f32)
    Wk = np.asarray(inputs["Wk"], dtype=f32)
    Wv = np.asarray(inputs["Wv"], dtype=f32)
    Wo = np.asarray(inputs["Wo"], dtype=f32)
    gW1 = np.asarray(inputs["gW1"], dtype=f32)
    gb1 = np.asarray(inputs["gb1"], dtype=f32)
    gW2 = np.asarray(inputs["gW2"], dtype=f32)
    gb2 = np.asarray(inputs["gb2"], dtype=f32)

    XT8 = _q8(X.T)                                       # [4096, 2048]
    # pre-tile to [partition, quarter, k-tile, 512] so every DMA moves
    # large contiguous per-partition segments
    XTT = np.ascontiguousarray(
        XT8.reshape(KT_TILES, 128, IC, 512).transpose(1, 2, 0, 3))

    def _tile_w(w8):  # [K, M] -> [128, K/128, M]
        kt = w8.shape[0] // 128
        return np.ascontiguousarray(
            w8.reshape(kt, 128, w8.shape[1]).transpose(1, 0, 2))
    # Wo row permutation to match the paired per-head AllGather chunk
    # assembly: OT_full row (p*2048 + (2r+u)*128 + d) holds global head
    # (4r + 2p + u), dim d.
    perm = np.empty(HID, dtype=np.int64)
    for p in range(2):
        for r in range(NC_):
            for u in range(2):
                g = 4 * r + 2 * p + u
                dst = p * 2048 + (2 * r + u) * 128
                perm[dst:dst + 128] = np.arange(g * 128, (g + 1) * 128)
    Wo_p = Wo[perm]
    mask01_t = np.ascontiguousarray(
        mask.astype(f32).reshape(JT, 128).T).astype(ml_dtypes.bfloat16)
    diagm = _q8(1.0 - np.eye(128, dtype=f32))

    in_maps = []
    for c in range(NC_):
        hsl = slice(c * HS, (c + 1) * HS)
        gsl = slice(c * GS, (c + 1) * GS)
        in_maps.append({
            "xt": XTT,
            "xshard": _tile_w(XT8[c * HS:(c + 1) * HS]),
            "wq": _tile_w(_q8(Wq[:, hsl], W_SCALE)),
            "wk": _tile_w(_q8(Wk[:, hsl], W_SCALE)),
            "wv": _tile_w(_q8(Wv[:, hsl], W_SCALE)),
            "wo": _tile_w(_q8(Wo_p[:, hsl], W_SCALE)),
            "gw1xr": _tile_w(_q8(gW1[c * HS:(c + 1) * HS], W_SCALE)),
            "gw1c": _tile_w(_q8(gW1[HID + c * HS:HID + (c + 1) * HS], W_SCALE)),
            "gw2": _tile_w(_q8(gW2[:, hsl], W_SCALE)),
            "gb1": np.ascontiguousarray(gb1[gsl].reshape(GS, 1)),
            "gb2": np.ascontiguousarray(gb2[hsl].reshape(4, 128).T),
            "mask01": mask01_t,
            "diagm": diagm,
        })
    return in_maps


_NC_CACHE = {}


def _run(inputs, trace=False):
    allones = bool(np.asarray(inputs["attention_mask"]).all())
    nc = _NC_CACHE.get(allones)
    if nc is None:
        nc = _build_program(allones)
        _NC_CACHE[allones] = nc
    in_maps = _make_in_maps(inputs)
    res = bass_utils.run_bass_kernel_spmd(
        nc, in_maps, core_ids=list(range(NC_)), trace=trace
    )
    shards = [np.asarray(res.results[c]["out"], dtype=np.float32)
              for c in range(NC_)]
    gated = np.concatenate(shards, axis=0).T  # gate * cross, [2048, 4096]
    out = np.asarray(inputs["hidden_states"], dtype=np.float32) + gated
    return np.ascontiguousarray(out), res


def kernel(**inputs) -> np.ndarray:
    out, _ = _run(inputs, trace=False)
    return out


# revision 18
# speedup vs baseline: 1.0869x; 1.0741x over previous
"""CrossBatchAttention Trainium2 kernel — 8-core tensor-parallel SPMD.

v3: same numerics as v2 (fp8 DoubleRow everywhere), restructured schedule:

  - Merged K/V/Q projection pass: one streaming sweep over the 4 X^T
    quarters computes kt, v and qt together (X loaded once, not 3x).
    The sync DMA queue is free of X traffic during the block phase.
  - Fine-grained startup: quarter-0 X and Wk arrive in 4-k-tile chunks
    consumed by a k-outer loop over 4 live PSUM banks, so the first
    matmul issues as soon as the first 256KB lands.
  - Paired OT AllGathers: heads {0,1} and {2,3} of each batch quarter
    share one AllGather (128KB in / 1MB out) -> 8 collectives instead
    of 16, halving CC-core occupancy.
  - Tight tail: outproj lags its AG by 2 blocks, chunk-3's g1c/RS is
    issued right after the last outproj, and collective triggers are
    ordered so the CC FIFO never parks a ready collective behind an
    unready one.

Quantization (unchanged from v2): X fp8, W* fp8 x64 (/64 on PSUM exit),
qt/kt bf16, P = exp(s/sqrt(d) - 5.0) fp8, ones = 1/8 so rec = 8/den,
otc = O*rec fp8, cacc fp8, g1 partials fp8 x8 through the RS,
sigmoid(logits/64). Host: concat 8 [512,2048] bf16 shards, transpose,
add X -> f32.
"""

import numpy as np
import ml_dtypes

import concourse.bass as bass
import concourse.mybir as mybir
import concourse.tile as tile
from concourse import bacc
from concourse import bass_utils

BF16 = mybir.dt.bfloat16
F32 = mybir.dt.float32
F8 = mybir.dt.float8e4
DR = mybir.MatmulPerfMode.DoubleRow
W_SCALE = 64.0           # all fp8 weights scaled by this on host
O_SCALE = 8.0            # otc = O * 8 (via ones=1/8 in denominator)
G_SCALE = 8.0            # g1 partials carried x8 through the RS
EBIAS = -5.0             # exp(s*SCALE + EBIAS): keeps P in fp8 range

B = 2048
HID = 4096
NH = 32
HD = 128
GH = 1024
NC_ = 8
HPC = NH // NC_          # heads per core = 4
HS = HID // NC_          # hid shard = 512
GS = GH // NC_           # gate-hidden shard = 128
SCALE = 1.0 / float(np.sqrt(HD))

KT_TILES = HID // 128    # 32 k-tiles over the 4096 contraction
KP = KT_TILES // 2       # 16 DoubleRow k-steps
JT = B // 128            # 16 j-tiles over keys
JP = JT // 2             # 8 DoubleRow j-steps
IC = B // 512            # 4 i-chunks of 512 over batch

GELU_FUNC = mybir.ActivationFunctionType.Gelu


def _build_program(allones: bool):
    nc = bacc.Bacc(
        "TRN2",
        target_bir_lowering=False,
        debug=False,
        enable_asserts=False,
        num_devices=NC_,
    )

    # ---- I/O declarations (per-core shapes) ----
    xt_d = nc.dram_tensor("xt", [128, IC, KT_TILES, 512], F8, kind="ExternalInput").ap()
    wq_d = nc.dram_tensor("wq", [128, KT_TILES, HS], F8, kind="ExternalInput").ap()
    wk_d = nc.dram_tensor("wk", [128, KT_TILES, HS], F8, kind="ExternalInput").ap()
    wv_d = nc.dram_tensor("wv", [128, KT_TILES, HS], F8, kind="ExternalInput").ap()
    wo_d = nc.dram_tensor("wo", [128, KT_TILES, HS], F8, kind="ExternalInput").ap()
    # gate W1, X part: full 4096 contraction x this core's 128 gh columns
    gw1x_d = nc.dram_tensor("gw1x", [128, KT_TILES, GS], F8, kind="ExternalInput").ap()
    # fused Wo @ gW1c (attn-output features -> gh), rows in AG perm order
    gwf_d = nc.dram_tensor("gwf", [128, KT_TILES, GS], F8, kind="ExternalInput").ap()
    gw2_d = nc.dram_tensor("gw2", [128, NC_, HS], F8, kind="ExternalInput").ap()
    gb1_d = nc.dram_tensor("gb1", [GS, 1], F32, kind="ExternalInput").ap()
    gb2_d = nc.dram_tensor("gb2", [128, 4], F32, kind="ExternalInput").ap()
    mask01_d = nc.dram_tensor("mask01", [128, JT], BF16, kind="ExternalInput").ap()
    diagm_d = nc.dram_tensor("diagm", [128, 128], F8, kind="ExternalInput").ap()
    out_d = nc.dram_tensor("out", [HS, B], BF16, kind="ExternalOutput").ap()

    groups = [list(range(NC_))]

    with tile.TileContext(nc) as tc:
        with (
            tc.tile_pool(name="persist", bufs=1) as persist,
            tc.tile_pool(name="psum", bufs=1, space="PSUM") as psum,
            tc.tile_pool(name="dram", bufs=1, space="DRAM") as dram,
        ):
            # ---------- persistent SBUF ----------
            qt_sb = persist.tile([128, HPC, B], BF16)     # [d, head, i] 2MB
            kt_sb = persist.tile([128, HPC, B], BF16)     # 2MB
            v_sb = persist.tile([128, JT, HS], F8)        # [j_in, j_tile, hd] 1MB
            mask01_sb = persist.tile([128, JT], BF16)
            diagm_sb = persist.tile([128, 128], F8)
            ones_sb = persist.tile([128, 2, 128], F8)
            ebias_sb = persist.tile([128, 1], F32)
            gb1_sb = persist.tile([GS, 1], F32)
            gb2_sb = persist.tile([128, 4], F32)
            # weights that live through the block phase
            wo_sb = persist.tile([128, KT_TILES, HS], F8)     # 2MB
            gw1x_sb = persist.tile([128, KT_TILES, GS], F8)
            gwf_sb = persist.tile([128, KT_TILES, GS], F8)
            gw2_sb = persist.tile([128, NC_, HS], F8)
            cacc = persist.tile([128, 4, B], F8)
            g1x_sb = persist.tile([128, B], F8)           # gW1x^T X, gh shard

            nc.vector.memset(ones_sb, 1.0 / O_SCALE)
            nc.vector.memset(ebias_sb, EBIAS)

            # ---------- DRAM bounce buffers for collectives ----------
            # paired OT AllGather: rank buffer [2, 128, 512] (heads 2p,2p+1)
            ag_in = [[None] * IC for _ in range(2)]
            ag_out = [[None] * IC for _ in range(2)]
            for p in range(2):
                for q in range(IC):
                    ag_in[p][q] = dram.tile([2, 128, 512], F8,
                                            name=f"ag_in{p}_{q}")
                    ag_out[p][q] = dram.tile(
                        [NC_ * 256, 512], F8, addr_space="Shared",
                        name=f"ag_out{p}_{q}"
                    )
            ag2_in_c, ag2_out_c = [], []
            for icc in range(IC):
                ag2_in_c.append(dram.tile([GS, 512], F8, name=f"ag2_in{icc}"))
                ag2_out_c.append(dram.tile([GH, 512], F8, addr_space="Shared",
                                           name=f"ag2_out{icc}"))

            # warmups with the same shapes as the real collectives so the
            # first real op doesn't pay the cold-path cost
            warm_ag_i = dram.tile([2, 128, 512], F8)
            warm_ag_o = dram.tile([NC_ * 256, 512], F8, addr_space="Shared")
            warm_ag2_i = dram.tile([GS, 512], F8)
            warm_ag2_o = dram.tile([GH, 512], F8, addr_space="Shared")
            nc.gpsimd.collective_compute(
                "AllGather", mybir.AluOpType.bypass, replica_groups=groups,
                ins=[warm_ag_i.opt()], outs=[warm_ag_o.opt()],
            )
            nc.gpsimd.collective_compute(
                "AllGather", mybir.AluOpType.bypass, replica_groups=groups,
                ins=[warm_ag2_i.opt()], outs=[warm_ag2_o.opt()],
            )

            with tc.tile_pool(name="main", bufs=1) as mp:
                # ======== merged K/V/Q projection pass ========
                with tc.tile_pool(name="pkvq", bufs=1) as pkvq:
                    wk_sb = pkvq.tile([128, KT_TILES, HS], F8, tag="wk", bufs=1)
                    wv_sb = pkvq.tile([128, KT_TILES, HS], F8, tag="wv", bufs=1)
                    wq_sb = pkvq.tile([128, KT_TILES, HS], F8, tag="wq", bufs=1)

                    # quarter-0 X and Wk in 4-k-tile chunks (256KB each),
                    # interleaved so the k-outer loop starts ASAP
                    xt_first = pkvq.tile([128, KT_TILES, 512], F8, tag="xt",
                                         bufs=2, name="xt_q")
                    NCH = 8   # chunks of 4 k-tiles
                    for ch in range(NCH):
                        ksl = slice(ch * 4, (ch + 1) * 4)
                        nc.sync.dma_start(out=xt_first[:, ksl, :],
                                          in_=xt_d[:, 0, ksl, :])
                        nc.sync.dma_start(out=wk_sb[:, ksl, :],
                                          in_=wk_d[:, ksl, :])
                    nc.sync.dma_start(out=wv_sb, in_=wv_d)

                    def load_xt(q):
                        xt_q = pkvq.tile([128, KT_TILES, 512], F8, tag="xt",
                                         bufs=2, name="xt_q")
                        nc.sync.dma_start(out=xt_q, in_=xt_d[:, q])
                        return xt_q

                    xt_next = load_xt(1)

                    # remaining weights on the scalar DMA queue so they don't
                    # block the projection-pass X streaming on the sync queue
                    nc.scalar.dma_start(out=wq_sb, in_=wq_d)
                    nc.scalar.dma_start(out=gw1x_sb, in_=gw1x_d)
                    nc.scalar.dma_start(out=wo_sb, in_=wo_d)
                    nc.scalar.dma_start(out=gwf_sb, in_=gwf_d)
                    nc.scalar.dma_start(out=gw2_sb, in_=gw2_d)
                    if not allones:
                        nc.scalar.dma_start(out=mask01_sb, in_=mask01_d)
                    nc.scalar.dma_start(out=diagm_sb, in_=diagm_d)
                    nc.scalar.dma_start(out=gb1_sb, in_=gb1_d)
                    nc.scalar.dma_start(out=gb2_sb, in_=gb2_d)

                    def proj_dr(wsb, msl, xt_q, ps):
                        for k in range(KP):
                            nc.tensor.matmul(
                                ps,
                                lhsT=wsb[:, 2 * k:2 * k + 2, msl],
                                rhs=xt_q[:, 2 * k:2 * k + 2, :],
                                start=(k == 0),
                                stop=(k == KP - 1),
                                perf_mode=DR,
                            )

                    def v_pass(q, xt_q):
                        for it in range(4):
                            ps = psum.tile([128, 512], F32, tag="mm", bufs=2,
                                           name="ps_v")
                            for k in range(KP):
                                nc.tensor.matmul(
                                    ps,
                                    lhsT=xt_q[:, 2 * k:2 * k + 2,
                                              it * 128:(it + 1) * 128],
                                    rhs=wv_sb[:, 2 * k:2 * k + 2, :],
                                    start=(k == 0),
                                    stop=(k == KP - 1),
                                    perf_mode=DR,
                                )
                            nc.vector.tensor_scalar_mul(
                                v_sb[:, q * 4 + it, :], ps, 1.0 / W_SCALE
                            )

                    def q_pass(q, xt_q):
                        isl = slice(q * 512, (q + 1) * 512)
                        for m in range(4):
                            ps = psum.tile([128, 512], F32, tag="mm", bufs=2,
                                           name="ps_q")
                            proj_dr(wq_sb, slice(m * 128, (m + 1) * 128),
                                    xt_q, ps)
                            nc.vector.tensor_scalar_mul(
                                qt_sb[:, m, isl], ps, 1.0 / W_SCALE
                            )

                    def g1x_pass(q, xt_q):
                        # gW1x^T X for this core's gh shard, full contraction
                        isl = slice(q * 512, (q + 1) * 512)
                        ps = psum.tile([128, 512], F32, tag="mm", bufs=2,
                                       name="ps_g1x")
                        proj_dr(gw1x_sb, slice(0, GS), xt_q, ps)
                        nc.vector.tensor_scalar_mul(
                            g1x_sb[:, isl], ps, 1.0 / W_SCALE
                        )

                    # --- quarter 0: k-outer K pass over 4 live PSUM banks ---
                    # (borrow the scores' "st" tag banks; they're idle here)
                    kpsA = psum.tile([128, 2, 512], F32, tag="st", bufs=2,
                                     name="kpsA")
                    kpsB = psum.tile([128, 2, 512], F32, tag="st", bufs=2,
                                     name="kpsB")
                    kps = [kpsA[:, 0, :], kpsA[:, 1, :],
                           kpsB[:, 0, :], kpsB[:, 1, :]]
                    for ch in range(NCH):
                        for m in range(4):
                            for u in range(2):
                                k = ch * 2 + u
                                nc.tensor.matmul(
                                    kps[m],
                                    lhsT=wk_sb[:, 4 * ch + 2 * u:
                                               4 * ch + 2 * u + 2,
                                               m * 128:(m + 1) * 128],
                                    rhs=xt_first[:, 4 * ch + 2 * u:
                                                 4 * ch + 2 * u + 2, :],
                                    start=(ch == 0 and u == 0),
                                    stop=(ch == NCH - 1 and u == 1),
                                    perf_mode=DR,
                                )
                    for m in range(4):
                        nc.vector.tensor_scalar_mul(
                            kt_sb[:, m, 0:512], kps[m], 1.0 / W_SCALE
                        )
                    v_pass(0, xt_first)
                    q_pass(0, xt_first)
                    g1x_pass(0, xt_first)

                    # --- quarters 1..3: standard m-outer loops ---
                    for q in range(1, IC):
                        xt_q = xt_next
                        if q + 1 < IC:
                            xt_next = load_xt(q + 1)
                        isl = slice(q * 512, (q + 1) * 512)
                        for m in range(4):
                            ps = psum.tile([128, 512], F32, tag="mm", bufs=2,
                                           name="ps_k")
                            proj_dr(wk_sb, slice(m * 128, (m + 1) * 128),
                                    xt_q, ps)
                            nc.vector.tensor_scalar_mul(
                                kt_sb[:, m, isl], ps, 1.0 / W_SCALE
                            )
                        v_pass(q, xt_q)
                        q_pass(q, xt_q)
                        g1x_pass(q, xt_q)

                # ======== interleaved block phase ========
                # block s: (h, q) = (s % 4, s // 4)

                def attention_block(h, q):
                    p = h // 2
                    u = h % 2
                    qsl = slice(q * 512, (q + 1) * 512)
                    pt = mp.tile([128, JT, 512], F8, tag="pt", bufs=2,
                                 name="pt")
                    for jp in range(JP):
                        st = psum.tile([128, 2, 512], F32, tag="st",
                                       bufs=2, name="st")
                        for uu in range(2):
                            j = 2 * jp + uu
                            nc.tensor.matmul(
                                st[:, uu, :],
                                lhsT=kt_sb[:, h, j * 128:(j + 1) * 128],
                                rhs=qt_sb[:, h, qsl],
                                start=True,
                                stop=True,
                            )
                        nc.scalar.activation(
                            pt[:, 2 * jp:2 * jp + 2, :],
                            st,
                            mybir.ActivationFunctionType.Exp,
                            bias=ebias_sb,
                            scale=SCALE,
                        )
                        for uu in range(2):
                            j = 2 * jp + uu
                            if not allones:
                                nc.vector.tensor_scalar_mul(
                                    pt[:, j, :], pt[:, j, :],
                                    mask01_sb[:, j:j + 1],
                                )
                            if j // 4 == q:
                                c0 = (j % 4) * 128
                                nc.vector.tensor_mul(
                                    pt[:, j, c0:c0 + 128],
                                    pt[:, j, c0:c0 + 128],
                                    diagm_sb,
                                )
                    den_ps = psum.tile([128, 512], F32, tag="acc", bufs=2,
                                       name="den_ps")
                    for jp in range(JP):
                        nc.tensor.matmul(
                            den_ps,
                            lhsT=ones_sb,
                            rhs=pt[:, 2 * jp:2 * jp + 2, :],
                            start=(jp == 0),
                            stop=(jp == JP - 1),
                            perf_mode=DR,
                        )
                    ot_ps = psum.tile([128, 512], F32, tag="acc", bufs=2,
                                      name="ot_ps")
                    for jp in range(JP):
                        nc.tensor.matmul(
                            ot_ps,
                            lhsT=v_sb[:, 2 * jp:2 * jp + 2,
                                      h * 128:(h + 1) * 128],
                            rhs=pt[:, 2 * jp:2 * jp + 2, :],
                            start=(jp == 0),
                            stop=(jp == JP - 1),
                            perf_mode=DR,
                        )
                    rec = mp.tile([128, 512], F32, tag="rec", bufs=1)
                    nc.vector.reciprocal_approx_fast(out=rec, in_=den_ps)
                    otc = mp.tile([128, 512], F8, tag="otc", bufs=1)
                    nc.vector.tensor_mul(otc, ot_ps, rec)
                    nc.sync.dma_start(out=ag_in[p][q][u], in_=otc)
                    if u == 1:
                        nc.gpsimd.collective_compute(
                            "AllGather",
                            mybir.AluOpType.bypass,
                            replica_groups=groups,
                            ins=[ag_in[p][q].opt()],
                            outs=[ag_out[p][q].opt()],
                        )

                def otg_load(p, ic):
                    otg = mp.tile([128, 2 * NC_, 512], F8, tag="otg", bufs=2,
                                  name="otg")
                    nc.sync.dma_start(
                        out=otg,
                        in_=ag_out[p][ic].rearrange("(g j) i -> j g i", j=128),
                    )
                    return otg

                def outproj_pair(p, ic, otg):
                    # contraction over the 16 gathered head-tiles of pair p
                    csl = slice(ic * 512, (ic + 1) * 512)
                    for m in range(4):
                        ps = psum.tile([128, 512], F32, tag="mm", bufs=2,
                                       name="ps_wo")
                        for r in range(NC_):
                            nc.tensor.matmul(
                                ps,
                                lhsT=wo_sb[:, p * 16 + 2 * r:
                                           p * 16 + 2 * r + 2,
                                           m * 128:(m + 1) * 128],
                                rhs=otg[:, 2 * r:2 * r + 2, :],
                                start=(r == 0),
                                stop=(r == NC_ - 1),
                                perf_mode=DR,
                            )
                        if p == 0:
                            nc.vector.tensor_scalar_mul(
                                cacc[:, m, csl], ps, 1.0 / (W_SCALE * O_SCALE)
                            )
                        else:
                            nc.vector.scalar_tensor_tensor(
                                cacc[:, m, csl], ps, 1.0 / (W_SCALE * O_SCALE),
                                cacc[:, m, csl],
                                op0=mybir.AluOpType.mult,
                                op1=mybir.AluOpType.add,
                            )

                def g1_chunk(ic, otg_a, otg_b):
                    # this core's gh-shard of g1 for the chunk, full local
                    # contraction: gWf^T @ otg (= gW1c^T cross) + g1X;
                    # then gelu and AllGather of the activated shard.
                    csl = slice(ic * 512, (ic + 1) * 512)
                    ps = psum.tile([128, 512], F32, tag="mm", bufs=2,
                                   name="ps_g1")
                    for r in range(NC_):
                        nc.tensor.matmul(
                            ps,
                            lhsT=gwf_sb[:, 2 * r:2 * r + 2, :],
                            rhs=otg_a[:, 2 * r:2 * r + 2, :],
                            start=(r == 0),
                            stop=False,
                            perf_mode=DR,
                        )
                    for r in range(NC_):
                        nc.tensor.matmul(
                            ps,
                            lhsT=gwf_sb[:, 16 + 2 * r:16 + 2 * r + 2, :],
                            rhs=otg_b[:, 2 * r:2 * r + 2, :],
                            start=False,
                            stop=(r == NC_ - 1),
                            perf_mode=DR,
                        )
                    g1pre = mp.tile([128, 512], BF16, tag="g1pre", bufs=1)
                    nc.vector.scalar_tensor_tensor(
                        g1pre, ps, 1.0 / (W_SCALE * O_SCALE),
                        g1x_sb[:, csl],
                        op0=mybir.AluOpType.mult,
                        op1=mybir.AluOpType.add,
                    )
                    gt_ch = mp.tile([128, 512], F8, tag="gt", bufs=1)
                    nc.scalar.activation(gt_ch, g1pre, GELU_FUNC,
                                         bias=gb1_sb, scale=1.0)
                    nc.sync.dma_start(out=ag2_in_c[ic], in_=gt_ch)
                    nc.gpsimd.collective_compute(
                        "AllGather",
                        mybir.AluOpType.bypass,
                        replica_groups=groups,
                        ins=[ag2_in_c[ic].opt()],
                        outs=[ag2_out_c[ic].opt()],
                    )

                def gtf_load(ic):
                    gtf = mp.tile([128, NC_, 512], F8, tag="gtf",
                                  bufs=1, name="gtf")
                    nc.sync.dma_start(
                        out=gtf,
                        in_=ag2_out_c[ic].rearrange("(r p) i -> p r i", p=128),
                    )
                    return gtf

                def gate_chain(ic, gtf):
                    csl = slice(ic * 512, (ic + 1) * 512)
                    for m in range(4):
                        ps = psum.tile([128, 512], F32, tag="mm", bufs=2,
                                       name="ps_gw2")
                        for r in range(NC_ // 2):
                            nc.tensor.matmul(
                                ps,
                                lhsT=gw2_sb[:, 2 * r:2 * r + 2,
                                            m * 128:(m + 1) * 128],
                                rhs=gtf[:, 2 * r:2 * r + 2, :],
                                start=(r == 0),
                                stop=(r == NC_ // 2 - 1),
                                perf_mode=DR,
                            )
                        gate_ch = mp.tile([128, 512], BF16, tag="gate",
                                          bufs=2)
                        nc.scalar.activation(
                            gate_ch, ps,
                            mybir.ActivationFunctionType.Sigmoid,
                            bias=gb2_sb[:, m:m + 1], scale=1.0 / W_SCALE,
                        )
                        outt = mp.tile([128, 512], BF16, tag="outt", bufs=1)
                        nc.vector.tensor_mul(outt, gate_ch, cacc[:, m, csl])
                        nc.sync.dma_start(
                            out=out_d[m * 128:(m + 1) * 128, csl], in_=outt
                        )

                # schedule:
                #   otg_A(ic) @ 4ic+2   outproj_A(ic) @ 4ic+3
                #   otg_B(ic) @ 4ic+4   outproj_B + g1_chunk(ic) @ 4ic+5
                #   gtf(ic) @ 4ic+7     gate(ic) @ 4ic+8
                otg_pend = {}
                gtf_pend = {}
                for s in range(16):
                    h, q = s % 4, s // 4
                    attention_block(h, q)
                    r4 = s % 4
                    if r4 == 2:
                        otg_pend[(0, q)] = otg_load(0, q)
                    if r4 == 0 and s >= 4:
                        otg_pend[(1, q - 1)] = otg_load(1, q - 1)
                    if r4 == 3:
                        outproj_pair(0, q, otg_pend[(0, q)])
                    if r4 == 1 and s >= 5:
                        ic = q - 1
                        otg_a = otg_pend.pop((0, ic))
                        otg_b = otg_pend.pop((1, ic))
                        outproj_pair(1, ic, otg_b)
                        g1_chunk(ic, otg_a, otg_b)
                    if r4 == 3 and s >= 7:
                        ic = (s - 7) // 4
                        gtf_pend[ic] = gtf_load(ic)
                    if r4 == 0 and s >= 8:
                        ic = (s - 8) // 4
                        gate_chain(ic, gtf_pend.pop(ic))

                # ---- tail: chunk 3 pair-B chain + chunk 2/3 gate chains ----
                otg_pend[(1, 3)] = otg_load(1, 3)
                outproj_pair(1, 3, otg_pend[(1, 3)])
                g1_chunk(3, otg_pend.pop((0, 3)), otg_pend.pop((1, 3)))
                gate_chain(2, gtf_pend.pop(2))
                gtf_pend[3] = gtf_load(3)
                gate_chain(3, gtf_pend.pop(3))

    nc.compile()
    return nc


def _q8(x, scale=1.0):
    f8 = ml_dtypes.float8_e4m3
    return np.ascontiguousarray(
        np.clip(np.asarray(x, dtype=np.float32) * scale, -240.0, 240.0)
    ).astype(f8)


def _make_in_maps(inputs):
    f32 = np.float32
    X = np.asarray(inputs["hidden_states"], dtype=f32)
    mask = np.asarray(inputs["attention_mask"])
    Wq = np.asarray(inputs["Wq"], dtype=f32)
    Wk = np.asarray(inputs["Wk"], dtype=f32)
    Wv = np.asarray(inputs["Wv"], dtype=f32)
    Wo = np.asarray(inputs["Wo"], dtype=f32)
    gW1 = np.asarray(inputs["gW1"], dtype=f32)
    gb1 = np.asarray(inputs["gb1"], dtype=f32)
    gW2 = np.asarray(inputs["gW2"], dtype=f32)
    gb2 = np.asarray(inputs["gb2"], dtype=f32)

    XT8 = _q8(X.T)                                       # [4096, 2048]
    # pre-tile to [partition, quarter, k-tile, 512] so every DMA moves
    # large contiguous per-partition segments
    XTT = np.ascontiguousarray(
        XT8.reshape(KT_TILES, 128, IC, 512).transpose(1, 2, 0, 3))

    def _tile_w(w8):  # [K, M] -> [128, K/128, M]
        kt = w8.shape[0] // 128
        return np.ascontiguousarray(
            w8.reshape(kt, 128, w8.shape[1]).transpose(1, 0, 2))
    # Wo row permutation to match the paired per-head AllGather chunk
    # assembly: OT_full row (p*2048 + (2r+u)*128 + d) holds global head
    # (4r + 2p + u), dim d.
    perm = np.empty(HID, dtype=np.int64)
    for p in range(2):
        for r in range(NC_):
            for u in range(2):
                g = 4 * r + 2 * p + u
                dst = p * 2048 + (2 * r + u) * 128
                perm[dst:dst + 128] = np.arange(g * 128, (g + 1) * 128)
    Wo_p = Wo[perm]
    mask01_t = np.ascontiguousarray(
        mask.astype(f32).reshape(JT, 128).T).astype(ml_dtypes.bfloat16)
    diagm = _q8(1.0 - np.eye(128, dtype=f32))

    # fused Wo @ gW1c: attention-output features (AG perm order) -> gh
    Wf_p = Wo_p @ gW1[HID:]                              # [4096, 1024]
    gW1x = gW1[:HID]                                     # [4096, 1024]

    in_maps = []
    for c in range(NC_):
        hsl = slice(c * HS, (c + 1) * HS)
        gsl = slice(c * GS, (c + 1) * GS)
        in_maps.append({
            "xt": XTT,
            "wq": _tile_w(_q8(Wq[:, hsl], W_SCALE)),
            "wk": _tile_w(_q8(Wk[:, hsl], W_SCALE)),
            "wv": _tile_w(_q8(Wv[:, hsl], W_SCALE)),
            "wo": _tile_w(_q8(Wo_p[:, hsl], W_SCALE)),
            "gw1x": _tile_w(_q8(gW1x[:, gsl], W_SCALE)),
            "gwf": _tile_w(_q8(Wf_p[:, gsl], W_SCALE)),
            "gw2": _tile_w(_q8(gW2[:, hsl], W_SCALE)),
            "gb1": np.ascontiguousarray(gb1[gsl].reshape(GS, 1)),
            "gb2": np.ascontiguousarray(gb2[hsl].reshape(4, 128).T),
            "mask01": mask01_t,
            "diagm": diagm,
        })
    return in_maps


_NC_CACHE = {}


def _run(inputs, trace=False):
    allones = bool(np.asarray(inputs["attention_mask"]).all())
    nc = _NC_CACHE.get(allones)
    if nc is None:
        nc = _build_program(allones)
        _NC_CACHE[allones] = nc
    in_maps = _make_in_maps(inputs)
    res = bass_utils.run_bass_kernel_spmd(
        nc, in_maps, core_ids=list(range(NC_)), trace=trace
    )
    shards = [np.asarray(res.results[c]["out"], dtype=np.float32)
              for c in range(NC_)]
    gated = np.concatenate(shards, axis=0).T  # gate * cross, [2048, 4096]
    out = np.asarray(inputs["hidden_states"], dtype=np.float32) + gated
    return np.ascontiguousarray(out), res


def kernel(**inputs) -> np.ndarray:
    out, _ = _run(inputs, trace=False)
    return out
